# revision 12
# baseline (speedup 1.0000x reference)
"""Trainium2 Bass kernel for nn_Attention: single-head attention,
B=32, N=1024, DIM=512, fp32 in/out.

    q = X @ Wq.T ; k = X @ Wk.T ; v = X @ Wv.T
    out = softmax((q k^T)/sqrt(D)) @ v

Strategy (8 NeuronCores, data-parallel over batch, 4 batches/core):
  - Host folds A = (Wq.T @ Wk)/sqrt(D) so scores = X A X.T — saves one
    projection-sized matmul per batch.
  - All matmul operands are bf16 (PSUM accumulation fp32): fp32r
    stationary loads cost ~224ns and gate the PE at ~272ns/matmul;
    bf16 loads are ~116ns and hide under the 213ns moving stream.
  - PSUM tiles are [128,1024] pairs (2 banks) so each ScalarE exp /
    eviction covers two 512-wide matmul groups: halves ScalarE op
    count.  PSUM: 2x ws-pair + 2x ot-pair = 8 banks; the softmax
    denominator tile shares the ws rotation.
  - Softmax along partitions: exp on ScalarE (bf16 out), partial sums
    on VectorE, denominator broadcast to all partitions via an
    all-ones [128,128] stationary matmul accumulated in three chunks
    (sum of exp chunks 0-5, then exp chunks 6 and 7 directly) so the
    PE never waits on the vector chain.  1/den = exp(-ln(den)) (both
    in one activation table).  Normalization multiplies read the
    attention accumulators straight out of PSUM (no eviction).
  - Phase-1 (G = (XA)^T and V) of batch b+1 is sliced into 8 pair-
    groups used as fillers between phase-2 stages of batch b, so
    single-rotation PSUM WAR gaps are always covered; the last batch
    uses dummy scratch matmuls as fillers.
  - Tile puts a tick-semaphore increment on every PE instruction
    (~26ns each); _coalesce_tick_updates defers increments within
    wait-free runs onto accumulation-group stops.
  - Warmup matmuls at t=0 keep the PE HAM clock-gate warming while
    the first input DMAs land (else the first ~15us run at 1.2GHz).
"""
import numpy as np

B, N, D = 32, 1024, 512
NCORES = 8
BPC = B // NCORES          # batches per core
DC = D // 128              # 4 chunks of 128 along d / e
KC = N // 128              # 8 chunks of 128 along k
QH = N // 512              # 2 q-halves of 512

_cache = {}


def _split_sync_waits(nc):
    """walrus on this image accepts at most ONE semaphore wait per
    instruction; hoist extras onto InstNoOp carriers on the same engine
    (same-engine program order preserves the gating)."""
    import concourse.mybir as mybir

    ctr = 0
    for f in nc.m.functions:
        for bb in f.blocks:
            out = []
            changed = False
            for ins in bb.instructions:
                si = getattr(ins, "sync_info", None)
                waits = list(si.on_wait) if si and si.on_wait else []
                if len(waits) > 1:
                    for w in waits[:-1]:
                        ctr += 1
                        out.append(
                            mybir.InstNoOp(
                                name=f"wsplit-{ctr}",
                                engine=ins.engine,
                                bass_nofuse=True,
                                sync_info=mybir.SyncInfo(on_wait=[w], on_update=[]),
                            )
                        )
                    ins.sync_info = mybir.SyncInfo(
                        on_wait=waits[-1:], on_update=list(si.on_update or [])
                    )
                    changed = True
                out.append(ins)
            if changed:
                bb.instructions[:] = out


def _coalesce_tick_updates(nc):
    """Every PE matmul carries a +1 update on the engine tick semaphore
    (~26ns of EVT_SEM write each).  Within a run of consecutive PE
    matmuls that carry no waits, defer the increments onto the run's
    boundary instructions (accumulation-group stops / the instruction
    before the next wait), summing the values.  Waits elsewhere observe
    the same cumulative counts no later than the carrier's retirement,
    and carriers are wait-free so they always retire: no deadlock."""
    import concourse.mybir as mybir

    for f in nc.m.functions:
        for bb in f.blocks:
            pend = 0
            carrier = None       # last stripped instruction
            carrier_upd = None   # its (stripped) SyncUpdate

            def flush():
                nonlocal pend, carrier, carrier_upd
                if carrier is not None and pend > 0:
                    carrier_upd.update_value = pend
                    carrier.sync_info = mybir.SyncInfo(
                        on_wait=list(carrier.sync_info.on_wait or []),
                        on_update=[carrier_upd],
                    )
                pend = 0
                carrier = None
                carrier_upd = None

            for ins in bb.instructions:
                if ins.engine != mybir.EngineType.PE:
                    continue
                si = getattr(ins, "sync_info", None)
                waits = list(si.on_wait) if si and si.on_wait else []
                ups = list(si.on_update) if si and si.on_update else []
                tick_only = (
                    type(ins).__name__ == "InstMatmult"
                    and len(ups) == 1
                    and ups[0].sync_type == "semaphore"
                    and ups[0].update_mode == "sem-inc"
                    and ups[0].update_reg is None
                )
                if waits:
                    flush()
                if not tick_only:
                    flush()
                    continue
                # strip the update, remember it
                ins.sync_info = mybir.SyncInfo(on_wait=waits, on_update=[])
                pend += ups[0].update_value
                carrier = ins
                carrier_upd = ups[0]
                if getattr(ins, "stop_tensor_calc", False):
                    flush()
            flush()


def _build():
    import concourse.bass as bass
    import concourse.mybir as mybir
    import concourse.tile as tile

    f32 = mybir.dt.float32
    bf16 = mybir.dt.bfloat16
    Exp = mybir.ActivationFunctionType.Exp
    Ln = mybir.ActivationFunctionType.Ln

    nc = bass.Bass(target_bir_lowering=False)

    xtb = nc.dram_tensor("xtb", [BPC, D, N], bf16, kind="ExternalInput")
    a_mat = nc.dram_tensor("a_mat", [D, D], bf16, kind="ExternalInput")
    wvt = nc.dram_tensor("wvt", [D, D], bf16, kind="ExternalInput")
    ones_bc_d = nc.dram_tensor("ones_bc", [128, 128], bf16, kind="ExternalInput")
    out_t = nc.dram_tensor("out_t", [BPC, D, N], f32, kind="ExternalOutput")

    with tile.TileContext(nc) as tc:
        with (
            tc.tile_pool(name="wpool", bufs=1) as wpool,
            tc.tile_pool(name="xpool", bufs=3) as xpool,
            tc.tile_pool(name="gpool", bufs=2) as gpool,
            tc.tile_pool(name="vpool", bufs=2) as vpool,
            tc.tile_pool(name="epool", bufs=3) as epool,
            tc.tile_pool(name="eapool", bufs=2) as eapool,
            tc.tile_pool(name="opool", bufs=2) as opool,
            tc.tile_pool(name="rpool", bufs=2) as rpool,
            tc.tile_pool(name="ps_ws", bufs=2, space="PSUM") as ps_ws,
            tc.tile_pool(name="ps_ot", bufs=2, space="PSUM") as ps_ot,
        ):
            # --- scratch for warmup + dummy fillers ---
            scr_st = wpool.tile([128, 128], bf16, tag="scr_st")
            scr_mv = wpool.tile([128, 512], bf16, tag="scr_mv")
            nc.vector.memset(scr_st[:], 0.0)
            nc.vector.memset(scr_mv[:], 0.0)

            # --- weights / constants; order DMAs so batch-0 needs land first ---
            a_sb = []
            for c in range(DC):
                t = wpool.tile([128, D], bf16, tag=f"a{c}", name=f"a_sb{c}")
                nc.sync.dma_start(t[:], a_mat[c * 128:(c + 1) * 128, :])
                a_sb.append(t)

            def load_xtb(b):
                xts = []
                for c in range(DC):
                    t = xpool.tile([128, N], bf16, tag=f"xt{c}", name=f"xt_b{b}c{c}")
                    nc.sync.dma_start(t[:], xtb[b, c * 128:(c + 1) * 128, :])
                    xts.append(t)
                return xts

            xts0 = load_xtb(0)
            ones_bc = wpool.tile([128, 128], bf16, tag="onebc")
            nc.sync.dma_start(ones_bc[:], ones_bc_d[:])
            wvt_sb = []
            for c in range(DC):
                t = wpool.tile([128, D], bf16, tag=f"wvt{c}", name=f"wvt_sb{c}")
                nc.sync.dma_start(t[:], wvt[c * 128:(c + 1) * 128, :])
                wvt_sb.append(t)
            xts1 = load_xtb(1)

            # --- warmup: PE busy from t=0 so HAM un-throttles before real work
            pw = ps_ws.tile([128, 1024], f32, tag="ws", name="warm_ps")
            for w in range(28):
                nc.tensor.matmul(
                    pw[:, 0:256], scr_st[:], scr_mv[:, 0:256],
                    start=True, stop=True,
                )

            def g_group(b, m, xts, gt_sb):
                """G^T pair-group: both q-halves for one e-chunk m."""
                pg = ps_ws.tile([128, 1024], f32, tag="ws", name=f"pg{b}{m}")
                for k4 in range(DC):
                    for h in range(QH):
                        nc.tensor.matmul(
                            pg[:, h * 512:(h + 1) * 512],
                            a_sb[k4][:, m * 128:(m + 1) * 128],
                            xts[k4][:, h * 512:(h + 1) * 512],
                            start=(k4 == 0), stop=(k4 == DC - 1),
                            skip_group_check=True,
                        )
                nc.scalar.copy(gt_sb[:, m * N:m * N + 1024], pg[:])

            def v_group(b, j, xts, v_sb):
                """V pair-group: key-chunks 2j and 2j+1."""
                pv = ps_ws.tile([128, 1024], f32, tag="ws", name=f"pv{b}{j}")
                for k4 in range(DC):
                    for half in range(2):
                        kc = 2 * j + half
                        nc.tensor.matmul(
                            pv[:, half * 512:(half + 1) * 512],
                            xts[k4][:, kc * 128:(kc + 1) * 128],
                            wvt_sb[k4][:],
                            start=(k4 == 0), stop=(k4 == DC - 1),
                            skip_group_check=True,
                        )
                nc.scalar.copy(v_sb[:, 2 * j * 512:(2 * j + 2) * 512], pv[:])

            def ph2(b, h, xts, gt_sb, v_sb, fillers):
                """Phase-2 for one (batch, q-half): scores, softmax,
                attention output, normalization, output DMA.  `fillers`
                is a list of 4 callables (phase-1 pair-groups of the
                next batch) slotted between stages to cover PSUM WAR
                rotation gaps; missing ones become dummy matmuls."""
                p_ot = [ps_ot.tile([128, 1024], f32, tag="ot",
                                   name=f"p_ot{b}{h}{i}") for i in range(2)]
                es = []
                p_den = [None]

                def dummy8(target):
                    for _ in range(8):
                        nc.tensor.matmul(target, scr_st[:], scr_mv[:],
                                         start=True, stop=True,
                                         skip_group_check=True)

                def fill(i):
                    if i < len(fillers):
                        fillers[i]()
                    elif i == 0:
                        # only the s0->s1 WAR gap needs covering when there
                        # is no real phase-1 work left (last batch)
                        dummy8(p_ot[0][:, 0:512])

                def s_pair(p):
                    p_st = ps_ws.tile([128, 1024], f32, tag="ws",
                                      name=f"st{b}{h}{p}")
                    for half in range(2):
                        kc = 2 * p + half
                        for k4 in range(DC):
                            nc.tensor.matmul(
                                p_st[:, half * 512:(half + 1) * 512],
                                xts[k4][:, kc * 128:(kc + 1) * 128],
                                gt_sb[:, k4 * N + h * 512:k4 * N + (h + 1) * 512],
                                start=(k4 == 0), stop=(k4 == DC - 1),
                                skip_group_check=True,
                            )
                    e_sb = epool.tile([128, 1024], bf16, tag="e",
                                      name=f"e{b}{h}{p}")
                    nc.scalar.activation(e_sb[:], p_st[:], Exp)
                    es.append(e_sb)

                def o_pair(p):
                    for half in range(2):
                        kc = 2 * p + half
                        for m in range(DC):
                            nc.tensor.matmul(
                                p_ot[m // 2][:, (m % 2) * 512:(m % 2 + 1) * 512],
                                v_sb[:, kc * 512 + m * 128:kc * 512 + (m + 1) * 128],
                                es[p][:, half * 512:(half + 1) * 512],
                                start=(kc == 0), stop=(kc == KC - 1),
                                skip_group_check=True,
                            )

                s_pair(0)
                fill(0)
                s_pair(1)
                o_pair(0)
                # running softmax-denominator partials on VectorE (bf16)
                ea01 = eapool.tile([128, 1024], bf16, tag="ea01", name=f"ea01_{b}{h}")
                nc.vector.tensor_add(ea01[:], es[0][:], es[1][:])
                s_pair(2)
                o_pair(1)
                ea012 = eapool.tile([128, 1024], bf16, tag="ea012", name=f"ea012_{b}{h}")
                nc.vector.tensor_add(ea012[:], ea01[:], es[2][:])
                easum = eapool.tile([128, 512], bf16, tag="easum", name=f"eas{b}{h}")
                nc.vector.tensor_add(easum[:], ea012[:, 0:512], ea012[:, 512:1024])
                # fillers sit well before the block tail so their evictions
                # finish before the next block reuses the ws rotation
                fill(1)
                fill(2)
                s_pair(3)
                o_pair(2)
                # den = sum_k E[k,q] broadcast to all partitions via all-ones
                # stationary, accumulated in 3 chunks; the last two share
                # o_pair(3)'s exp dependency so the PE never stalls here.
                pd = ps_ws.tile([128, 1024], f32, tag="ws", name=f"den{b}{h}")
                p_den[0] = pd
                nc.tensor.matmul(pd[:, 0:512], ones_bc[:], easum[:],
                                 start=True, stop=False, skip_group_check=True)
                o_pair(3)
                nc.tensor.matmul(pd[:, 0:512], ones_bc[:], es[3][:, 0:512],
                                 start=False, stop=False, skip_group_check=True)
                nc.tensor.matmul(pd[:, 0:512], ones_bc[:], es[3][:, 512:1024],
                                 start=False, stop=True, skip_group_check=True)
                # normalization chain (off PE): 1/den = exp(-ln(den))
                ln_sb = rpool.tile([128, 512], f32, tag="ln", name=f"ln{b}{h}")
                nc.scalar.activation(ln_sb[:], pd[:, 0:512], Ln)
                rc_sb = rpool.tile([128, 512], f32, tag="rc", name=f"rc{b}{h}")
                nc.scalar.activation(rc_sb[:], ln_sb[:], Exp, scale=-1.0)
                ot_sb = opool.tile([128, DC * 512], f32, tag="ot", name=f"osb{b}{h}")
                for m in range(DC):
                    nc.vector.tensor_mul(
                        ot_sb[:, m * 512:(m + 1) * 512],
                        p_ot[m // 2][:, (m % 2) * 512:(m % 2 + 1) * 512],
                        rc_sb[:],
                    )
                    nc.sync.dma_start(
                        out_t[b, m * 128:(m + 1) * 128, h * 512:(h + 1) * 512],
                        ot_sb[:, m * 512:(m + 1) * 512],
                    )
                fill(3)

            # --- batch 0 phase-1 standalone (ws rotation covers the gaps) ---
            gt0 = gpool.tile([128, DC * N], bf16, tag="gt", name="gt_b0")
            for m in range(DC):
                g_group(0, m, xts0, gt0)
            v0 = vpool.tile([128, KC * 512], bf16, tag="v", name="v_b0")
            for j in range(KC // 2):
                v_group(0, j, xts0, v0)

            xts = {0: xts0, 1: xts1}
            gt = {0: gt0}
            v = {0: v0}
            for b in range(BPC):
                if b + 2 < BPC:
                    xts[b + 2] = load_xtb(b + 2)
                fillers_g = []
                fillers_v = []
                if b + 1 < BPC:
                    gt[b + 1] = gpool.tile([128, DC * N], bf16, tag="gt",
                                           name=f"gt_b{b+1}")
                    v[b + 1] = vpool.tile([128, KC * 512], bf16, tag="v",
                                          name=f"v_b{b+1}")
                    fillers_g = [
                        (lambda m=m: g_group(b + 1, m, xts[b + 1], gt[b + 1]))
                        for m in range(DC)
                    ]
                    fillers_v = [
                        (lambda j=j: v_group(b + 1, j, xts[b + 1], v[b + 1]))
                        for j in range(KC // 2)
                    ]
                ph2(b, 0, xts[b], gt[b], v[b], fillers_g)
                ph2(b, 1, xts[b], gt[b], v[b], fillers_v)
    return nc


def _prepare_inputs(embeddings, Wq, Wk, Wv):
    import ml_dtypes

    bf16 = ml_dtypes.bfloat16
    xt_all = np.ascontiguousarray(
        embeddings.transpose(0, 2, 1)
    ).astype(bf16)
    a_mat = (
        Wq.astype(np.float64).T @ Wk.astype(np.float64) / np.sqrt(float(D))
    ).astype(bf16)
    wvt = np.ascontiguousarray(Wv.T).astype(bf16)
    ones_bc = np.ones((128, 128), bf16)
    in_maps = []
    for i in range(NCORES):
        in_maps.append(
            {
                "xtb": np.ascontiguousarray(xt_all[i * BPC:(i + 1) * BPC]),
                "a_mat": a_mat,
                "wvt": wvt,
                "ones_bc": ones_bc,
            }
        )
    return in_maps


def _get_nc():
    if "nc" not in _cache:
        import os

        nc = _build()
        if not os.environ.get("BASS_NO_COALESCE"):
            _coalesce_tick_updates(nc)
        _split_sync_waits(nc)
        _cache["nc"] = nc
    return _cache["nc"]


def _assemble(results):
    out = np.empty((B, N, D), np.float32)
    for i in range(NCORES):
        ot = results[i]["out_t"]  # [BPC, D, N]
        out[i * BPC:(i + 1) * BPC] = ot.transpose(0, 2, 1)
    return out


def kernel(embeddings, Wq, Wk, Wv):
    from concourse.bass_utils import run_bass_kernel_spmd

    embeddings = np.asarray(embeddings, dtype=np.float32)
    in_maps = _prepare_inputs(
        embeddings, np.asarray(Wq), np.asarray(Wk), np.asarray(Wv)
    )
    res = run_bass_kernel_spmd(_get_nc(), in_maps, list(range(NCORES)))
    return _assemble(res.results)


# revision 16
# speedup vs baseline: 1.1950x; 1.1950x over previous
"""Trainium2 Bass kernel for nn_Attention: single-head attention,
B=32, N=1024, DIM=512, fp32 in/out.

    q = X @ Wq.T ; k = X @ Wk.T ; v = X @ Wv.T
    out = softmax((q k^T)/sqrt(D)) @ v

Strategy (8 NeuronCores, data-parallel over batch, 4 batches/core):
  - Host folds A = (Wq.T @ Wk)/sqrt(D) so scores = X A X.T — saves one
    projection-sized matmul per batch.
  - All matmul operands are bf16 (PSUM accumulation fp32): fp32r
    stationary loads cost ~224ns and gate the PE at ~272ns/matmul;
    bf16 loads are ~116ns and hide under the 213ns moving stream.
  - PSUM tiles are [128,1024] pairs (2 banks) so each ScalarE exp /
    eviction covers two 512-wide matmul groups: halves ScalarE op
    count.  PSUM: 2x ws-pair + 2x ot-pair = 8 banks; the softmax
    denominator tile shares the ws rotation.
  - Softmax along partitions: exp on ScalarE (bf16 out), partial sums
    on VectorE, denominator broadcast to all partitions via an
    all-ones [128,128] stationary matmul accumulated in three chunks
    (sum of exp chunks 0-5, then exp chunks 6 and 7 directly) so the
    PE never waits on the vector chain.  1/den = exp(-ln(den)) (both
    in one activation table).  Normalization multiplies read the
    attention accumulators straight out of PSUM (no eviction).
  - Phase-1 (G = (XA)^T and V) of batch b+1 is sliced into 8 pair-
    groups used as fillers between phase-2 stages of batch b, so
    single-rotation PSUM WAR gaps are always covered; the last batch
    uses dummy scratch matmuls as fillers.
  - Warmup matmuls at t=0 keep the PE HAM clock-gate warming while
    the first input DMAs land (else the first ~15us run at 1.2GHz).
"""
import numpy as np

B, N, D = 32, 1024, 512
NCORES = 8
BPC = B // NCORES          # batches per core
DC = D // 128              # 4 chunks of 128 along d / e
KC = N // 128              # 8 chunks of 128 along k
QH = N // 512              # 2 q-halves of 512

_cache = {}


def _split_sync_waits(nc):
    """walrus on this image accepts at most ONE semaphore wait per
    instruction; hoist extras onto InstNoOp carriers on the same engine
    (same-engine program order preserves the gating)."""
    import concourse.mybir as mybir

    ctr = 0
    for f in nc.m.functions:
        for bb in f.blocks:
            out = []
            changed = False
            for ins in bb.instructions:
                si = getattr(ins, "sync_info", None)
                waits = list(si.on_wait) if si and si.on_wait else []
                if len(waits) > 1:
                    for w in waits[:-1]:
                        ctr += 1
                        out.append(
                            mybir.InstNoOp(
                                name=f"wsplit-{ctr}",
                                engine=ins.engine,
                                bass_nofuse=True,
                                sync_info=mybir.SyncInfo(on_wait=[w], on_update=[]),
                            )
                        )
                    ins.sync_info = mybir.SyncInfo(
                        on_wait=waits[-1:], on_update=list(si.on_update or [])
                    )
                    changed = True
                out.append(ins)
            if changed:
                bb.instructions[:] = out


def _build():
    import concourse.bass as bass
    import concourse.mybir as mybir
    import concourse.tile as tile

    f32 = mybir.dt.float32
    bf16 = mybir.dt.bfloat16
    Exp = mybir.ActivationFunctionType.Exp
    Ln = mybir.ActivationFunctionType.Ln

    nc = bass.Bass(target_bir_lowering=False)

    xtb = nc.dram_tensor("xtb", [BPC, D, N], bf16, kind="ExternalInput")
    a_mat = nc.dram_tensor("a_mat", [D, D], bf16, kind="ExternalInput")
    wvt = nc.dram_tensor("wvt", [D, D], bf16, kind="ExternalInput")
    ones_bc_d = nc.dram_tensor("ones_bc", [128, 128], bf16, kind="ExternalInput")
    out_t = nc.dram_tensor("out_t", [BPC, D, N], f32, kind="ExternalOutput")

    with tile.TileContext(nc) as tc:
        with (
            tc.tile_pool(name="wpool", bufs=1) as wpool,
            tc.tile_pool(name="xpool", bufs=3) as xpool,
            tc.tile_pool(name="gpool", bufs=2) as gpool,
            tc.tile_pool(name="vpool", bufs=2) as vpool,
            tc.tile_pool(name="epool", bufs=3) as epool,
            tc.tile_pool(name="eapool", bufs=2) as eapool,
            tc.tile_pool(name="opool", bufs=2) as opool,
            tc.tile_pool(name="rpool", bufs=2) as rpool,
            tc.tile_pool(name="ps_ws", bufs=2, space="PSUM") as ps_ws,
            tc.tile_pool(name="ps_ot", bufs=2, space="PSUM") as ps_ot,
        ):
            # --- scratch for warmup + dummy fillers ---
            scr_st = wpool.tile([128, 128], bf16, tag="scr_st")
            scr_mv = wpool.tile([128, 512], bf16, tag="scr_mv")
            nc.vector.memset(scr_st[:], 0.0)
            nc.vector.memset(scr_mv[:], 0.0)

            # --- weights / constants; order DMAs so batch-0 needs land first ---
            a_sb = []
            for c in range(DC):
                t = wpool.tile([128, D], bf16, tag=f"a{c}", name=f"a_sb{c}")
                nc.sync.dma_start(t[:], a_mat[c * 128:(c + 1) * 128, :])
                a_sb.append(t)

            def load_xtb(b):
                xts = []
                for c in range(DC):
                    t = xpool.tile([128, N], bf16, tag=f"xt{c}", name=f"xt_b{b}c{c}")
                    nc.sync.dma_start(t[:], xtb[b, c * 128:(c + 1) * 128, :])
                    xts.append(t)
                return xts

            xts0 = load_xtb(0)
            ones_bc = wpool.tile([128, 128], bf16, tag="onebc")
            nc.sync.dma_start(ones_bc[:], ones_bc_d[:])
            wvt_sb = []
            for c in range(DC):
                t = wpool.tile([128, D], bf16, tag=f"wvt{c}", name=f"wvt_sb{c}")
                nc.sync.dma_start(t[:], wvt[c * 128:(c + 1) * 128, :])
                wvt_sb.append(t)
            xts1 = load_xtb(1)

            # --- warmup: PE busy from t=0 so HAM un-throttles before real work
            pw = ps_ws.tile([128, 1024], f32, tag="ws", name="warm_ps")
            for w in range(24):
                nc.tensor.matmul(
                    pw[:, 0:256], scr_st[:], scr_mv[:, 0:256],
                    start=True, stop=True,
                )

            def g_group(b, m, xts, gt_sb):
                """G^T pair-group: both q-halves for one e-chunk m."""
                pg = ps_ws.tile([128, 1024], f32, tag="ws", name=f"pg{b}{m}")
                for k4 in range(DC):
                    for h in range(QH):
                        nc.tensor.matmul(
                            pg[:, h * 512:(h + 1) * 512],
                            a_sb[k4][:, m * 128:(m + 1) * 128],
                            xts[k4][:, h * 512:(h + 1) * 512],
                            start=(k4 == 0), stop=(k4 == DC - 1),
                            skip_group_check=True,
                        )
                nc.scalar.copy(gt_sb[:, m * N:m * N + 1024], pg[:])

            def v_group(b, j, xts, v_sb):
                """V pair-group: key-chunks 2j and 2j+1."""
                pv = ps_ws.tile([128, 1024], f32, tag="ws", name=f"pv{b}{j}")
                for k4 in range(DC):
                    for half in range(2):
                        kc = 2 * j + half
                        nc.tensor.matmul(
                            pv[:, half * 512:(half + 1) * 512],
                            xts[k4][:, kc * 128:(kc + 1) * 128],
                            wvt_sb[k4][:],
                            start=(k4 == 0), stop=(k4 == DC - 1),
                            skip_group_check=True,
                        )
                nc.scalar.copy(v_sb[:, 2 * j * 512:(2 * j + 2) * 512], pv[:])

            def ph2(b, h, xts, gt_sb, v_sb, fillers):
                """Phase-2 for one (batch, q-half): scores, softmax,
                attention output, normalization, output DMA.  `fillers`
                is a list of 4 callables (phase-1 pair-groups of the
                next batch) slotted between stages to cover PSUM WAR
                rotation gaps; missing ones become dummy matmuls."""
                p_ot = [ps_ot.tile([128, 1024], f32, tag="ot",
                                   name=f"p_ot{b}{h}{i}") for i in range(2)]
                es = []
                p_den = [None]

                def dummy8(target):
                    for _ in range(8):
                        nc.tensor.matmul(target, scr_st[:], scr_mv[:],
                                         start=True, stop=True,
                                         skip_group_check=True)

                def fill(i):
                    if i < len(fillers):
                        fillers[i]()
                    elif i == 0:
                        # only the s0->s1 WAR gap needs covering when there
                        # is no real phase-1 work left (last batch)
                        dummy8(p_ot[0][:, 0:512])

                def s_pair(p):
                    p_st = ps_ws.tile([128, 1024], f32, tag="ws",
                                      name=f"st{b}{h}{p}")
                    for half in range(2):
                        kc = 2 * p + half
                        for k4 in range(DC):
                            nc.tensor.matmul(
                                p_st[:, half * 512:(half + 1) * 512],
                                xts[k4][:, kc * 128:(kc + 1) * 128],
                                gt_sb[:, k4 * N + h * 512:k4 * N + (h + 1) * 512],
                                start=(k4 == 0), stop=(k4 == DC - 1),
                                skip_group_check=True,
                            )
                    e_sb = epool.tile([128, 1024], bf16, tag="e",
                                      name=f"e{b}{h}{p}")
                    nc.scalar.activation(e_sb[:], p_st[:], Exp)
                    es.append(e_sb)

                def o_pair(p):
                    for half in range(2):
                        kc = 2 * p + half
                        for m in range(DC):
                            nc.tensor.matmul(
                                p_ot[m // 2][:, (m % 2) * 512:(m % 2 + 1) * 512],
                                v_sb[:, kc * 512 + m * 128:kc * 512 + (m + 1) * 128],
                                es[p][:, half * 512:(half + 1) * 512],
                                start=(kc == 0), stop=(kc == KC - 1),
                                skip_group_check=True,
                            )

                s_pair(0)
                fill(0)
                s_pair(1)
                o_pair(0)
                # running softmax-denominator partials on VectorE (bf16)
                ea01 = eapool.tile([128, 1024], bf16, tag="ea01", name=f"ea01_{b}{h}")
                nc.vector.tensor_add(ea01[:], es[0][:], es[1][:])
                s_pair(2)
                o_pair(1)
                ea012 = eapool.tile([128, 1024], bf16, tag="ea012", name=f"ea012_{b}{h}")
                nc.vector.tensor_add(ea012[:], ea01[:], es[2][:])
                easum = eapool.tile([128, 512], bf16, tag="easum", name=f"eas{b}{h}")
                nc.vector.tensor_add(easum[:], ea012[:, 0:512], ea012[:, 512:1024])
                # fillers sit well before the block tail so their evictions
                # finish before the next block reuses the ws rotation
                fill(1)
                fill(2)
                s_pair(3)
                o_pair(2)
                # den = sum_k E[k,q] broadcast to all partitions via all-ones
                # stationary, accumulated in 3 chunks; the last two share
                # o_pair(3)'s exp dependency so the PE never stalls here.
                pd = ps_ws.tile([128, 1024], f32, tag="ws", name=f"den{b}{h}")
                p_den[0] = pd
                nc.tensor.matmul(pd[:, 0:512], ones_bc[:], easum[:],
                                 start=True, stop=False, skip_group_check=True)
                o_pair(3)
                nc.tensor.matmul(pd[:, 0:512], ones_bc[:], es[3][:, 0:512],
                                 start=False, stop=False, skip_group_check=True)
                nc.tensor.matmul(pd[:, 0:512], ones_bc[:], es[3][:, 512:1024],
                                 start=False, stop=True, skip_group_check=True)
                # normalization chain (off PE): 1/den = exp(-ln(den))
                ln_sb = rpool.tile([128, 512], f32, tag="ln", name=f"ln{b}{h}")
                nc.scalar.activation(ln_sb[:], pd[:, 0:512], Ln)
                rc_sb = rpool.tile([128, 512], f32, tag="rc", name=f"rc{b}{h}")
                nc.scalar.activation(rc_sb[:], ln_sb[:], Exp, scale=-1.0)
                ot_sb = opool.tile([128, DC * 512], f32, tag="ot", name=f"osb{b}{h}")
                for m in range(DC):
                    nc.vector.tensor_mul(
                        ot_sb[:, m * 512:(m + 1) * 512],
                        p_ot[m // 2][:, (m % 2) * 512:(m % 2 + 1) * 512],
                        rc_sb[:],
                    )
                    nc.sync.dma_start(
                        out_t[b, m * 128:(m + 1) * 128, h * 512:(h + 1) * 512],
                        ot_sb[:, m * 512:(m + 1) * 512],
                    )
                fill(3)

            # --- batch 0 phase-1 standalone (ws rotation covers the gaps) ---
            gt0 = gpool.tile([128, DC * N], bf16, tag="gt", name="gt_b0")
            for m in range(DC):
                g_group(0, m, xts0, gt0)
            v0 = vpool.tile([128, KC * 512], bf16, tag="v", name="v_b0")
            for j in range(KC // 2):
                v_group(0, j, xts0, v0)

            xts = {0: xts0, 1: xts1}
            gt = {0: gt0}
            v = {0: v0}
            for b in range(BPC):
                if b + 2 < BPC:
                    xts[b + 2] = load_xtb(b + 2)
                fillers_g = []
                fillers_v = []
                if b + 1 < BPC:
                    gt[b + 1] = gpool.tile([128, DC * N], bf16, tag="gt",
                                           name=f"gt_b{b+1}")
                    v[b + 1] = vpool.tile([128, KC * 512], bf16, tag="v",
                                          name=f"v_b{b+1}")
                    fillers_g = [
                        (lambda m=m: g_group(b + 1, m, xts[b + 1], gt[b + 1]))
                        for m in range(DC)
                    ]
                    fillers_v = [
                        (lambda j=j: v_group(b + 1, j, xts[b + 1], v[b + 1]))
                        for j in range(KC // 2)
                    ]
                ph2(b, 0, xts[b], gt[b], v[b], fillers_g)
                ph2(b, 1, xts[b], gt[b], v[b], fillers_v)
    return nc


def _prepare_inputs(embeddings, Wq, Wk, Wv):
    import ml_dtypes

    bf16 = ml_dtypes.bfloat16
    xt_all = np.ascontiguousarray(
        embeddings.transpose(0, 2, 1)
    ).astype(bf16)
    a_mat = (
        Wq.astype(np.float64).T @ Wk.astype(np.float64) / np.sqrt(float(D))
    ).astype(bf16)
    wvt = np.ascontiguousarray(Wv.T).astype(bf16)
    ones_bc = np.ones((128, 128), bf16)
    in_maps = []
    for i in range(NCORES):
        in_maps.append(
            {
                "xtb": np.ascontiguousarray(xt_all[i * BPC:(i + 1) * BPC]),
                "a_mat": a_mat,
                "wvt": wvt,
                "ones_bc": ones_bc,
            }
        )
    return in_maps


def _get_nc():
    if "nc" not in _cache:
        nc = _build()
        _split_sync_waits(nc)
        _cache["nc"] = nc
    return _cache["nc"]


def _assemble(results):
    out = np.empty((B, N, D), np.float32)
    for i in range(NCORES):
        ot = results[i]["out_t"]  # [BPC, D, N]
        out[i * BPC:(i + 1) * BPC] = ot.transpose(0, 2, 1)
    return out


def kernel(embeddings, Wq, Wk, Wv):
    from concourse.bass_utils import run_bass_kernel_spmd

    embeddings = np.asarray(embeddings, dtype=np.float32)
    in_maps = _prepare_inputs(
        embeddings, np.asarray(Wq), np.asarray(Wk), np.asarray(Wv)
    )
    res = run_bass_kernel_spmd(_get_nc(), in_maps, list(range(NCORES)))
    return _assemble(res.results)


# revision 19
# speedup vs baseline: 1.1964x; 1.0012x over previous
"""Trainium2 Bass kernel for nn_Attention: single-head attention,
B=32, N=1024, DIM=512, fp32 in/out.

    q = X @ Wq.T ; k = X @ Wk.T ; v = X @ Wv.T
    out = softmax((q k^T)/sqrt(D)) @ v

Strategy (8 NeuronCores, data-parallel over batch, 4 batches/core):
  - Host folds A = (Wq.T @ Wk)/sqrt(D) so scores = X A X.T — saves one
    projection-sized matmul per batch.
  - All matmul operands are bf16 (PSUM accumulation fp32): fp32r
    stationary loads cost ~224ns and gate the PE at ~272ns/matmul;
    bf16 loads are ~116ns and hide under the 213ns moving stream.
  - PSUM tiles are [128,1024] pairs (2 banks) so each ScalarE exp /
    eviction covers two 512-wide matmul groups: halves ScalarE op
    count.  PSUM: 2x ws-pair + 2x ot-pair = 8 banks; the softmax
    denominator tile shares the ws rotation.
  - Softmax along partitions: exp on ScalarE (bf16 out), partial sums
    on VectorE, denominator broadcast to all partitions via an
    all-ones [128,128] stationary matmul accumulated in three chunks
    (sum of exp chunks 0-5, then exp chunks 6 and 7 directly) so the
    PE never waits on the vector chain.  1/den = exp(-ln(den)) (both
    in one activation table).  Normalization multiplies read the
    attention accumulators straight out of PSUM (no eviction).
  - Phase-1 (G = (XA)^T and V) of batch b+1 is sliced into 8 pair-
    groups used as fillers between phase-2 stages of batch b, so
    single-rotation PSUM WAR gaps are always covered; the last batch
    uses dummy scratch matmuls as fillers.
  - Warmup matmuls at t=0 keep the PE HAM clock-gate warming while
    the first input DMAs land (else the first ~15us run at 1.2GHz).
"""
import numpy as np

B, N, D = 32, 1024, 512
NCORES = 8
BPC = B // NCORES          # batches per core
DC = D // 128              # 4 chunks of 128 along d / e
KC = N // 128              # 8 chunks of 128 along k
QH = N // 512              # 2 q-halves of 512

_cache = {}


def _split_sync_waits(nc):
    """walrus on this image accepts at most ONE semaphore wait per
    instruction; hoist extras onto InstNoOp carriers on the same engine
    (same-engine program order preserves the gating)."""
    import concourse.mybir as mybir

    ctr = 0
    for f in nc.m.functions:
        for bb in f.blocks:
            out = []
            changed = False
            for ins in bb.instructions:
                si = getattr(ins, "sync_info", None)
                waits = list(si.on_wait) if si and si.on_wait else []
                if len(waits) > 1:
                    for w in waits[:-1]:
                        ctr += 1
                        out.append(
                            mybir.InstNoOp(
                                name=f"wsplit-{ctr}",
                                engine=ins.engine,
                                bass_nofuse=True,
                                sync_info=mybir.SyncInfo(on_wait=[w], on_update=[]),
                            )
                        )
                    ins.sync_info = mybir.SyncInfo(
                        on_wait=waits[-1:], on_update=list(si.on_update or [])
                    )
                    changed = True
                out.append(ins)
            if changed:
                bb.instructions[:] = out


def _build():
    import concourse.bass as bass
    import concourse.mybir as mybir
    import concourse.tile as tile

    f32 = mybir.dt.float32
    bf16 = mybir.dt.bfloat16
    Exp = mybir.ActivationFunctionType.Exp
    Ln = mybir.ActivationFunctionType.Ln

    nc = bass.Bass(target_bir_lowering=False)

    xtb = nc.dram_tensor("xtb", [BPC, D, N], bf16, kind="ExternalInput")
    a_mat = nc.dram_tensor("a_mat", [D, D], bf16, kind="ExternalInput")
    wvt = nc.dram_tensor("wvt", [D, D], bf16, kind="ExternalInput")
    ones_bc_d = nc.dram_tensor("ones_bc", [128, 128], bf16, kind="ExternalInput")
    out_t = nc.dram_tensor("out_t", [BPC, D, N], f32, kind="ExternalOutput")

    with tile.TileContext(nc) as tc:
        with (
            tc.tile_pool(name="wpool", bufs=1) as wpool,
            tc.tile_pool(name="xpool", bufs=3) as xpool,
            tc.tile_pool(name="gpool", bufs=2) as gpool,
            tc.tile_pool(name="vpool", bufs=2) as vpool,
            tc.tile_pool(name="epool", bufs=3) as epool,
            tc.tile_pool(name="eapool", bufs=2) as eapool,
            tc.tile_pool(name="opool", bufs=2) as opool,
            tc.tile_pool(name="rpool", bufs=2) as rpool,
            tc.tile_pool(name="ps_ws", bufs=2, space="PSUM") as ps_ws,
            tc.tile_pool(name="ps_ot", bufs=2, space="PSUM") as ps_ot,
        ):
            # --- scratch for warmup + dummy fillers ---
            scr_st = wpool.tile([128, 128], bf16, tag="scr_st")
            scr_mv = wpool.tile([128, 512], bf16, tag="scr_mv")
            nc.vector.memset(scr_st[:], 0.0)
            nc.vector.memset(scr_mv[:], 0.0)

            # --- weights / constants; single wide DMA per tensor (issue
            # serialization on SP was costing ~3us at the head), ordered so
            # batch-0's needs land first ---
            a_all = wpool.tile([128, DC * D], bf16, tag="a_all")
            nc.sync.dma_start(
                a_all[:].rearrange("p (c e) -> p c e", c=DC),
                a_mat.rearrange("(c p) e -> p c e", p=128),
            )
            a_sb = [a_all[:, c * D:(c + 1) * D] for c in range(DC)]

            def load_xtb(b):
                t = xpool.tile([128, DC * N], bf16, tag="xt", name=f"xt_b{b}")
                nc.sync.dma_start(
                    t[:].rearrange("p (c n) -> p c n", c=DC),
                    xtb[b].rearrange("(c p) n -> p c n", p=128),
                )
                return [t[:, c * N:(c + 1) * N] for c in range(DC)]

            xts0 = load_xtb(0)
            ones_bc = wpool.tile([128, 128], bf16, tag="onebc")
            nc.sync.dma_start(ones_bc[:], ones_bc_d[:])
            wvt_all = wpool.tile([128, DC * D], bf16, tag="wvt_all")
            nc.sync.dma_start(
                wvt_all[:].rearrange("p (c e) -> p c e", c=DC),
                wvt.rearrange("(c p) e -> p c e", p=128),
            )
            wvt_sb = [wvt_all[:, c * D:(c + 1) * D] for c in range(DC)]
            xts1 = load_xtb(1)

            # --- warmup: PE busy from t=0 so HAM un-throttles before real work
            pw = ps_ws.tile([128, 1024], f32, tag="ws", name="warm_ps")
            for w in range(24):
                nc.tensor.matmul(
                    pw[:, 0:256], scr_st[:], scr_mv[:, 0:256],
                    start=True, stop=True,
                )

            def g_group(b, m, xts, gt_sb):
                """G^T pair-group: both q-halves for one e-chunk m."""
                pg = ps_ws.tile([128, 1024], f32, tag="ws", name=f"pg{b}{m}")
                for k4 in range(DC):
                    for h in range(QH):
                        nc.tensor.matmul(
                            pg[:, h * 512:(h + 1) * 512],
                            a_sb[k4][:, m * 128:(m + 1) * 128],
                            xts[k4][:, h * 512:(h + 1) * 512],
                            start=(k4 == 0), stop=(k4 == DC - 1),
                            skip_group_check=True,
                        )
                nc.scalar.copy(gt_sb[:, m * N:m * N + 1024], pg[:])

            def v_group(b, j, xts, v_sb):
                """V pair-group: key-chunks 2j and 2j+1."""
                pv = ps_ws.tile([128, 1024], f32, tag="ws", name=f"pv{b}{j}")
                for k4 in range(DC):
                    for half in range(2):
                        kc = 2 * j + half
                        nc.tensor.matmul(
                            pv[:, half * 512:(half + 1) * 512],
                            xts[k4][:, kc * 128:(kc + 1) * 128],
                            wvt_sb[k4][:],
                            start=(k4 == 0), stop=(k4 == DC - 1),
                            skip_group_check=True,
                        )
                nc.scalar.copy(v_sb[:, 2 * j * 512:(2 * j + 2) * 512], pv[:])

            def ph2(b, h, xts, gt_sb, v_sb, fillers):
                """Phase-2 for one (batch, q-half): scores, softmax,
                attention output, normalization, output DMA.  `fillers`
                is a list of 4 callables (phase-1 pair-groups of the
                next batch) slotted between stages to cover PSUM WAR
                rotation gaps; missing ones become dummy matmuls."""
                p_ot = [ps_ot.tile([128, 1024], f32, tag="ot",
                                   name=f"p_ot{b}{h}{i}") for i in range(2)]
                es = []
                p_den = [None]

                def dummy8(target):
                    for _ in range(8):
                        nc.tensor.matmul(target, scr_st[:], scr_mv[:],
                                         start=True, stop=True,
                                         skip_group_check=True)

                def fill(i):
                    if i < len(fillers):
                        fillers[i]()
                    elif i == 0:
                        # only the s0->s1 WAR gap needs covering when there
                        # is no real phase-1 work left (last batch)
                        dummy8(p_ot[0][:, 0:512])

                def s_pair(p):
                    p_st = ps_ws.tile([128, 1024], f32, tag="ws",
                                      name=f"st{b}{h}{p}")
                    for half in range(2):
                        kc = 2 * p + half
                        for k4 in range(DC):
                            nc.tensor.matmul(
                                p_st[:, half * 512:(half + 1) * 512],
                                xts[k4][:, kc * 128:(kc + 1) * 128],
                                gt_sb[:, k4 * N + h * 512:k4 * N + (h + 1) * 512],
                                start=(k4 == 0), stop=(k4 == DC - 1),
                                skip_group_check=True,
                            )
                    e_sb = epool.tile([128, 1024], bf16, tag="e",
                                      name=f"e{b}{h}{p}")
                    nc.scalar.activation(e_sb[:], p_st[:], Exp)
                    es.append(e_sb)

                def o_pair(p):
                    for half in range(2):
                        kc = 2 * p + half
                        for m in range(DC):
                            nc.tensor.matmul(
                                p_ot[m // 2][:, (m % 2) * 512:(m % 2 + 1) * 512],
                                v_sb[:, kc * 512 + m * 128:kc * 512 + (m + 1) * 128],
                                es[p][:, half * 512:(half + 1) * 512],
                                start=(kc == 0), stop=(kc == KC - 1),
                                skip_group_check=True,
                            )

                s_pair(0)
                fill(0)
                s_pair(1)
                o_pair(0)
                # running softmax-denominator partials on VectorE (bf16)
                ea01 = eapool.tile([128, 1024], bf16, tag="ea01", name=f"ea01_{b}{h}")
                nc.vector.tensor_add(ea01[:], es[0][:], es[1][:])
                s_pair(2)
                o_pair(1)
                ea012 = eapool.tile([128, 1024], bf16, tag="ea012", name=f"ea012_{b}{h}")
                nc.vector.tensor_add(ea012[:], ea01[:], es[2][:])
                easum = eapool.tile([128, 512], bf16, tag="easum", name=f"eas{b}{h}")
                nc.vector.tensor_add(easum[:], ea012[:, 0:512], ea012[:, 512:1024])
                # fillers sit well before the block tail so their evictions
                # finish before the next block reuses the ws rotation
                fill(1)
                fill(2)
                s_pair(3)
                o_pair(2)
                # den = sum_k E[k,q] broadcast to all partitions via all-ones
                # stationary, accumulated in 3 chunks; the last two share
                # o_pair(3)'s exp dependency so the PE never stalls here.
                pd = ps_ws.tile([128, 1024], f32, tag="ws", name=f"den{b}{h}")
                p_den[0] = pd
                nc.tensor.matmul(pd[:, 0:512], ones_bc[:], easum[:],
                                 start=True, stop=False, skip_group_check=True)
                o_pair(3)
                nc.tensor.matmul(pd[:, 0:512], ones_bc[:], es[3][:, 0:512],
                                 start=False, stop=False, skip_group_check=True)
                nc.tensor.matmul(pd[:, 0:512], ones_bc[:], es[3][:, 512:1024],
                                 start=False, stop=True, skip_group_check=True)
                # F3 before ln/rc so its eviction isn't queued behind them
                # on ScalarE (the next block's F0 slot reuses its ws buffer)
                fill(3)
                # normalization chain (off PE): 1/den = exp(-ln(den))
                ln_sb = rpool.tile([128, 512], f32, tag="ln", name=f"ln{b}{h}")
                nc.scalar.activation(ln_sb[:], pd[:, 0:512], Ln)
                rc_sb = rpool.tile([128, 512], f32, tag="rc", name=f"rc{b}{h}")
                nc.scalar.activation(rc_sb[:], ln_sb[:], Exp, scale=-1.0)
                ot_sb = opool.tile([128, DC * 512], f32, tag="ot", name=f"osb{b}{h}")
                for m in range(DC):
                    nc.vector.tensor_mul(
                        ot_sb[:, m * 512:(m + 1) * 512],
                        p_ot[m // 2][:, (m % 2) * 512:(m % 2 + 1) * 512],
                        rc_sb[:],
                    )
                    nc.sync.dma_start(
                        out_t[b, m * 128:(m + 1) * 128, h * 512:(h + 1) * 512],
                        ot_sb[:, m * 512:(m + 1) * 512],
                    )

            # --- batch 0 phase-1 standalone (ws rotation covers the gaps) ---
            gt0 = gpool.tile([128, DC * N], bf16, tag="gt", name="gt_b0")
            for m in range(DC):
                g_group(0, m, xts0, gt0)
            v0 = vpool.tile([128, KC * 512], bf16, tag="v", name="v_b0")
            for j in range(KC // 2):
                v_group(0, j, xts0, v0)

            xts = {0: xts0, 1: xts1}
            gt = {0: gt0}
            v = {0: v0}
            for b in range(BPC):
                if b + 2 < BPC:
                    xts[b + 2] = load_xtb(b + 2)
                fillers_g = []
                fillers_v = []
                if b + 1 < BPC:
                    gt[b + 1] = gpool.tile([128, DC * N], bf16, tag="gt",
                                           name=f"gt_b{b+1}")
                    v[b + 1] = vpool.tile([128, KC * 512], bf16, tag="v",
                                          name=f"v_b{b+1}")
                    fillers_g = [
                        (lambda m=m: g_group(b + 1, m, xts[b + 1], gt[b + 1]))
                        for m in range(DC)
                    ]
                    fillers_v = [
                        (lambda j=j: v_group(b + 1, j, xts[b + 1], v[b + 1]))
                        for j in range(KC // 2)
                    ]
                ph2(b, 0, xts[b], gt[b], v[b], fillers_g)
                ph2(b, 1, xts[b], gt[b], v[b], fillers_v)
    return nc


def _prepare_inputs(embeddings, Wq, Wk, Wv):
    import ml_dtypes

    bf16 = ml_dtypes.bfloat16
    xt_all = np.ascontiguousarray(
        embeddings.transpose(0, 2, 1)
    ).astype(bf16)
    a_mat = (
        Wq.astype(np.float64).T @ Wk.astype(np.float64) / np.sqrt(float(D))
    ).astype(bf16)
    wvt = np.ascontiguousarray(Wv.T).astype(bf16)
    ones_bc = np.ones((128, 128), bf16)
    in_maps = []
    for i in range(NCORES):
        in_maps.append(
            {
                "xtb": np.ascontiguousarray(xt_all[i * BPC:(i + 1) * BPC]),
                "a_mat": a_mat,
                "wvt": wvt,
                "ones_bc": ones_bc,
            }
        )
    return in_maps


def _get_nc():
    if "nc" not in _cache:
        nc = _build()
        _split_sync_waits(nc)
        _cache["nc"] = nc
    return _cache["nc"]


def _assemble(results):
    out = np.empty((B, N, D), np.float32)
    for i in range(NCORES):
        ot = results[i]["out_t"]  # [BPC, D, N]
        out[i * BPC:(i + 1) * BPC] = ot.transpose(0, 2, 1)
    return out


def kernel(embeddings, Wq, Wk, Wv):
    from concourse.bass_utils import run_bass_kernel_spmd

    embeddings = np.asarray(embeddings, dtype=np.float32)
    in_maps = _prepare_inputs(
        embeddings, np.asarray(Wq), np.asarray(Wk), np.asarray(Wv)
    )
    res = run_bass_kernel_spmd(_get_nc(), in_maps, list(range(NCORES)))
    return _assemble(res.results)
